# revision 1
# baseline (speedup 1.0000x reference)
"""Trainium2 Bass kernel for nn_CombinedRotaryEmbedding.

Math: the reference applies 32 sequential "blended Givens" column rotations
(each linear in x), then multiplies by r_matrix, then applies a RoPE-style
sin/cos mix per sequence position.  Every step is linear in x, so for each
position s the whole pipeline collapses to one 64x64 matrix:

    out_row(s) = x_row @ (B_1 B_2 ... B_32 @ r_matrix @ R_s) = x_row @ M_s

where R_s is the (sparse) RoPE rotation for position s.  We fold M_s on the
host in float64, shard the 4096 positions across the 8 cores (512 each; all
batches/heads per position = 128 rows), and the device kernel is a pure
stream of 64x64 fp32 matmuls: out.T[j, bh] = M_s.T @ x_s.T.  The x shards
are pre-transposed on the host so the contraction dim (feature) lands on
SBUF partitions, making every DMA fully contiguous; pairs of positions run
concurrently in the PE array via (0,0)/(64,64) tile_position packing.
"""

import numpy as np
from contextlib import ExitStack

import concourse.bass as bass
import concourse.mybir as mybir
import concourse.tile as tile
from concourse.bass_utils import run_bass_kernel_spmd

B, S, D = 8, 4096, 1024
HEAD, H_DIM, ROT = 16, 64, 32
N_CORES = 8
S_CORE = S // N_CORES          # 512 positions per core
GROUP_POS = 32                 # positions per device-loop iteration
N_GROUPS = S_CORE // GROUP_POS  # 16
HALF = GROUP_POS // 2          # 16 positions per partition-block
F32 = mybir.dt.float32


# ---------------------------------------------------------------- host math
def _fold_matrices(thetas, theta_scale, r_matrix, inv_freq, pairs):
    """Fold blended-Givens steps + r_matrix + per-position RoPE into M[s]."""
    th = (thetas.astype(np.float64) * np.float64(theta_scale[0]))
    E = np.eye(H_DIM, dtype=np.float64)
    for k in range(ROT):
        i, j = int(pairs[k, 0]), int(pairs[k, 1])
        c, s = np.cos(th[k]), np.sin(th[k])
        xi = E[:, i].copy()
        xj = E[:, j].copy()
        gi = xi * c + xj * s
        gj = -xi * s + xj * c
        E[:, i] = (2.0 * gi + xi - 2.0 * gi * c) / 3.0
        E[:, j] = (2.0 * gj + xj - 2.0 * gi * s) / 3.0
    A = E @ r_matrix.astype(np.float64)                      # [64, 64]

    pos = np.arange(S, dtype=np.float32)
    # match the reference: the angle product is computed in fp32
    sinu = (pos[:, None] * inv_freq[None, :].astype(np.float32)).astype(np.float32)
    c = np.cos(sinu.astype(np.float64))                      # [S, 32]
    s = np.sin(sinu.astype(np.float64))
    A1 = A[:, 0::2]                                          # [64, 32]
    A2 = A[:, 1::2]
    M = np.empty((S, H_DIM, H_DIM), dtype=np.float64)
    M[:, :, :ROT] = A1[None] * c[:, None, :] - A2[None] * s[:, None, :]
    M[:, :, ROT:] = A1[None] * s[:, None, :] + A2[None] * c[:, None, :]
    return M.astype(np.float32)                              # [S, 64, 64]


# ------------------------------------------------------------- bass program
def _split_multiwait(nc):
    """This walrus build rejects >1 sync wait per CTRL instruction; hoist
    extra waits from the Tile tail drain onto single-wait NOPs."""
    fn = nc.m.functions[0]
    for bb in fn.blocks:
        insts = list(bb.instructions)
        out, changed = [], False
        for inst in insts:
            si = getattr(inst, "sync_info", None)
            if si is not None and si.on_wait and len(si.on_wait) > 1:
                waits = list(si.on_wait)
                eng = nc.engines[inst.engine]
                for w in waits[:-1]:
                    ni = eng.nop().ins
                    for bb2 in fn.blocks:
                        cur = list(bb2.instructions)
                        if any(x.name == ni.name for x in cur):
                            bb2.instructions = [x for x in cur if x.name != ni.name]
                    si2 = ni.sync_info
                    if si2 is None:
                        ni.sync_info = mybir.SyncInfo(on_wait=[w], on_update=[])
                    else:
                        si2.on_wait = [w]
                        ni.sync_info = si2
                    out.append(ni)
                si.on_wait = [waits[-1]]
                inst.sync_info = si
                changed = True
            out.append(inst)
        if changed:
            bb.instructions = out


_NC_CACHE = {}


def _build_nc(repeats=1, bufs=3, skip_mm=False, skip_mdma=False):
    key = (repeats, bufs, skip_mm, skip_mdma)
    if key in _NC_CACHE:
        return _NC_CACHE[key]
    nc = bass.Bass()
    x_ext = nc.declare_dram_parameter("xin", [N_GROUPS, 128, HALF * 128], F32,
                                      isOutput=False)
    m_ext = nc.declare_dram_parameter("min", [N_GROUPS, 128, HALF * 64], F32,
                                      isOutput=False)
    y_ext = nc.declare_dram_parameter("yout", [128, S_CORE * 64], F32,
                                      isOutput=True)

    with tile.TileContext(nc) as tc, ExitStack() as ctx:
        xp = ctx.enter_context(tc.tile_pool(name="xp", bufs=bufs))
        mp = ctx.enter_context(tc.tile_pool(name="mp", bufs=bufs))
        op = ctx.enter_context(tc.tile_pool(name="op", bufs=bufs))
        pp = ctx.enter_context(tc.tile_pool(name="pp", bufs=8, space="PSUM"))

        for g in [g for _ in range(repeats) for g in range(N_GROUPS)]:
            xt = xp.tile([128, HALF * 128], F32)
            nc.sync.dma_start(xt[:], x_ext[g])
            if skip_mdma:  # diagnostic: 32 KB of M instead of 1 MB
                mt = mp.tile([128, 64], F32)
                nc.sync.dma_start(mt[:], m_ext[g][:, 0:64])
            else:
                mt = mp.tile([128, HALF * 64], F32)
                nc.sync.dma_start(mt[:], m_ext[g])
            if skip_mm:   # DMA-only diagnostic: bounce xt straight out
                nc.sync.dma_start(
                    y_ext[:, g * HALF * 128:(g + 1) * HALF * 128], xt[:])
                continue
            ot = op.tile([128, HALF * 128], F32)
            for q in range(4):                     # 4 psum banks per group
                ps = pp.tile([128, 512], F32)
                for i4 in range(4):
                    i = q * 4 + i4
                    mlo = i * 64 if not skip_mdma else 0
                    nc.tensor.matmul(
                        ps[0:64, i4 * 128:(i4 + 1) * 128],
                        lhsT=mt[0:64, mlo:mlo + 64],
                        rhs=xt[0:64, i * 128:(i + 1) * 128],
                        tile_position=(0, 0),
                    )
                    nc.tensor.matmul(
                        ps[64:128, i4 * 128:(i4 + 1) * 128],
                        lhsT=mt[64:128, mlo:mlo + 64],
                        rhs=xt[64:128, i * 128:(i + 1) * 128],
                        tile_position=(64, 64),
                    )
                if q % 2 == 0:
                    nc.scalar.copy(ot[:, q * 512:(q + 1) * 512], ps[:])
                else:
                    nc.vector.tensor_copy(ot[:, q * 512:(q + 1) * 512], ps[:])
            nc.sync.dma_start(
                y_ext[:, g * HALF * 128:(g + 1) * HALF * 128], ot[:])

    _split_multiwait(nc)
    _NC_CACHE[key] = nc
    return nc


# ----------------------------------------------------------------- wrapper
def _prep_inputs(x, M):
    """Per-core host shard/layout. Returns list of in_maps."""
    xr = x.reshape(B, S, HEAD, H_DIM)
    in_maps = []
    for c in range(N_CORES):
        xc = xr[:, c * S_CORE:(c + 1) * S_CORE]              # [8, 512, 16, 64]
        # pos_loc = 32 g + 16 blk + i  ->  axes (b, g, blk, i, h, f)
        xc = xc.reshape(B, N_GROUPS, 2, HALF, HEAD, H_DIM)
        xc = np.ascontiguousarray(xc.transpose(1, 2, 5, 3, 0, 4))
        xc = xc.reshape(N_GROUPS, 128, HALF * 128)
        mc = M[c * S_CORE:(c + 1) * S_CORE]                  # [512, 64, 64]
        mc = mc.reshape(N_GROUPS, 2, HALF, H_DIM, H_DIM)
        mc = np.ascontiguousarray(mc.transpose(0, 1, 3, 2, 4))
        mc = mc.reshape(N_GROUPS, 128, HALF * 64)
        in_maps.append({"xin": xc, "min": mc})
    return in_maps


def _gather_output(results):
    out = np.empty((B, S, HEAD, H_DIM), dtype=np.float32)
    for c in range(N_CORES):
        yc = results[c]["yout"]                              # [128, 512*64]
        yc = yc.reshape(2, H_DIM, N_GROUPS, HALF, B, HEAD)   # blk j g i b h
        yc = yc.transpose(4, 2, 0, 3, 5, 1)                  # b g blk i h j
        out[:, c * S_CORE:(c + 1) * S_CORE] = yc.reshape(B, S_CORE, HEAD, H_DIM)
    return out.reshape(B, S, D)


def kernel(x, thetas, theta_scale, r_matrix, inv_freq, pairs, **_unused):
    x = np.asarray(x, dtype=np.float32)
    M = _fold_matrices(np.asarray(thetas), np.asarray(theta_scale),
                       np.asarray(r_matrix), np.asarray(inv_freq),
                       np.asarray(pairs))
    nc = _build_nc()
    in_maps = _prep_inputs(x, M)
    res = run_bass_kernel_spmd(nc, in_maps, list(range(N_CORES)))
    return _gather_output(res.results)



# revision 2
# speedup vs baseline: 15.3981x; 15.3981x over previous
"""Trainium2 Bass kernel for nn_CombinedRotaryEmbedding.

Math: the reference applies 32 sequential "blended Givens" column rotations
(each linear in x), then multiplies by r_matrix, then applies a RoPE-style
sin/cos mix per sequence position.  Every step is linear in x, so for each
position s the whole pipeline collapses to one 64x64 matrix:

    out_row(s) = x_row @ (B_1 B_2 ... B_32 @ r_matrix @ R_s) = x_row @ M_s

where R_s is the (sparse) RoPE rotation for position s.  We fold M_s on the
host in float64, shard the 4096 positions across the 8 cores (512 each; all
batches/heads per position = 128 rows), and the device kernel is a pure
stream of 64x64 matmuls: out.T[j, bh] = M_s.T @ x_s.T.

The kernel is HBM-DMA-bound, so everything streams in fp16 (the output is
upcast to fp32 on the host; end-to-end error ~8e-4 against the fp32
reference): x 8.4 MB + M 4.2 MB in, y 8.4 MB out per core.  Schedule notes:
  - inputs stream in 3 large chunks (248/248/16 positions) so the DMA
    engines never idle; the small tail chunk keeps the final
    load->matmul->copy->store chain off the critical path;
  - output DMAs issue from the scalar engine: DMA waits hold the issuing
    sequencer, so putting stores on the same queue as loads stalls the
    input stream;
  - PSUM->SBUF copies (with fp32->fp16 cast) alternate scalar/vector;
  - positions pack two-per-PE-pass via (0,0)/(64,64) tile_position.
"""

import numpy as np
from contextlib import ExitStack

import concourse.bass as bass
import concourse.mybir as mybir
import concourse.tile as tile
from concourse.bass_utils import run_bass_kernel_spmd

B, S, D = 8, 4096, 1024
HEAD, H_DIM, ROT = 16, 64, 32
N_CORES = 8
S_CORE = S // N_CORES          # 512 positions per core
HC = S_CORE // 2               # 256 positions per partition-block
CHUNKS = (248, 248, 16)        # DMA chunk sizes (positions)
CG = 64                        # compute-group size (positions)
F32 = mybir.dt.float32
F16 = mybir.dt.float16


# ---------------------------------------------------------------- host math
def _fold_matrices(thetas, theta_scale, r_matrix, inv_freq, pairs):
    """Fold blended-Givens steps + r_matrix + per-position RoPE into M[s]."""
    th = (thetas.astype(np.float64) * np.float64(theta_scale[0]))
    E = np.eye(H_DIM, dtype=np.float64)
    for k in range(ROT):
        i, j = int(pairs[k, 0]), int(pairs[k, 1])
        c, s = np.cos(th[k]), np.sin(th[k])
        xi = E[:, i].copy()
        xj = E[:, j].copy()
        gi = xi * c + xj * s
        gj = -xi * s + xj * c
        E[:, i] = (2.0 * gi + xi - 2.0 * gi * c) / 3.0
        E[:, j] = (2.0 * gj + xj - 2.0 * gi * s) / 3.0
    A = E @ r_matrix.astype(np.float64)                      # [64, 64]

    pos = np.arange(S, dtype=np.float32)
    # match the reference: the angle product is computed in fp32
    sinu = (pos[:, None] * inv_freq[None, :].astype(np.float32)).astype(np.float32)
    c = np.cos(sinu.astype(np.float64))                      # [S, 32]
    s = np.sin(sinu.astype(np.float64))
    A1 = A[:, 0::2]                                          # [64, 32]
    A2 = A[:, 1::2]
    M = np.empty((S, H_DIM, H_DIM), dtype=np.float64)
    M[:, :, :ROT] = A1[None] * c[:, None, :] - A2[None] * s[:, None, :]
    M[:, :, ROT:] = A1[None] * s[:, None, :] + A2[None] * c[:, None, :]
    return M.astype(np.float16)                              # [S, 64, 64]


# ------------------------------------------------------------- bass program
def _split_multiwait(nc):
    """This walrus build rejects >1 sync wait per CTRL instruction; hoist
    extra waits from the Tile tail drain onto single-wait NOPs."""
    fn = nc.m.functions[0]
    for bb in fn.blocks:
        insts = list(bb.instructions)
        out, changed = [], False
        for inst in insts:
            si = getattr(inst, "sync_info", None)
            if si is not None and si.on_wait and len(si.on_wait) > 1:
                waits = list(si.on_wait)
                eng = nc.engines[inst.engine]
                for w in waits[:-1]:
                    ni = eng.nop().ins
                    for bb2 in fn.blocks:
                        cur = list(bb2.instructions)
                        if any(x.name == ni.name for x in cur):
                            bb2.instructions = [x for x in cur if x.name != ni.name]
                    si2 = ni.sync_info
                    if si2 is None:
                        ni.sync_info = mybir.SyncInfo(on_wait=[w], on_update=[])
                    else:
                        si2.on_wait = [w]
                        ni.sync_info = si2
                    out.append(ni)
                si.on_wait = [waits[-1]]
                inst.sync_info = si
                changed = True
            out.append(inst)
        if changed:
            bb.instructions = out


_NC_CACHE = {}


def _build_nc(repeats=1, bufs=2):
    """SBUF partitions 0:64 = features of core-positions 0:256, parts
    64:128 = positions 256:512.  x_ext/y_ext columns = (pos % 256) * 128
    + row;  m_ext columns = (pos % 256) * 64 + fo."""
    key = (repeats, bufs)
    if key in _NC_CACHE:
        return _NC_CACHE[key]
    nc = bass.Bass()
    x_ext = nc.declare_dram_parameter("xin", [128, HC * 128], F16,
                                      isOutput=False)
    m_ext = nc.declare_dram_parameter("min", [128, HC * 64], F16,
                                      isOutput=False)
    y_ext = nc.declare_dram_parameter("yout", [128, HC * 128], F16,
                                      isOutput=True)

    with tile.TileContext(nc) as tc, ExitStack() as ctx:
        xp = ctx.enter_context(tc.tile_pool(name="xp", bufs=bufs))
        mp = ctx.enter_context(tc.tile_pool(name="mp", bufs=bufs))
        op = ctx.enter_context(tc.tile_pool(name="op", bufs=4))
        pp = ctx.enter_context(tc.tile_pool(name="pp", bufs=8, space="PSUM"))

        for _ in range(repeats):
            qcnt = 0
            off = 0                       # position offset within half
            for ch in CHUNKS:
                h = ch // 2               # positions per half in this chunk
                xt = xp.tile([128, h * 128], F16)
                nc.sync.dma_start(xt[:], x_ext[:, off * 128:(off + h) * 128])
                mt = mp.tile([128, h * 64], F16)
                nc.sync.dma_start(mt[:], m_ext[:, off * 64:(off + h) * 64])
                for sub in range(0, h, CG // 2):   # compute groups
                    scg = min(CG // 2, h - sub)    # positions per half
                    ot = op.tile([128, scg * 128], F16)
                    for q in range(scg // 4):      # psum tiles: 4 pos/half
                        ps = pp.tile([128, 512], F32)
                        for i4 in range(4):
                            p = sub + q * 4 + i4
                            nc.tensor.matmul(
                                ps[0:64, i4 * 128:(i4 + 1) * 128],
                                lhsT=mt[0:64, p * 64:(p + 1) * 64],
                                rhs=xt[0:64, p * 128:(p + 1) * 128],
                                tile_position=(0, 0),
                            )
                            nc.tensor.matmul(
                                ps[64:128, i4 * 128:(i4 + 1) * 128],
                                lhsT=mt[64:128, p * 64:(p + 1) * 64],
                                rhs=xt[64:128, p * 128:(p + 1) * 128],
                                tile_position=(64, 64),
                            )
                        if qcnt % 2 == 0:
                            nc.scalar.copy(ot[:, q * 512:(q + 1) * 512], ps[:])
                        else:
                            nc.vector.tensor_copy(
                                ot[:, q * 512:(q + 1) * 512], ps[:])
                        qcnt += 1
                    # stores issue from the scalar engine: their sem waits
                    # must not block the input-DMA issue stream on sync
                    nc.scalar.dma_start(
                        y_ext[:, (off + sub) * 128:(off + sub + scg) * 128],
                        ot[:])
                off += h

    _split_multiwait(nc)
    _NC_CACHE[key] = nc
    return nc


# ----------------------------------------------------------------- wrapper
def _prep_inputs(x, M):
    """Per-core host shard/layout. Returns list of in_maps."""
    xr = x.astype(np.float16).reshape(B, S, HEAD, H_DIM)
    in_maps = []
    for c in range(N_CORES):
        xc = xr[:, c * S_CORE:(c + 1) * S_CORE]              # [8, 512, 16, 64]
        xc = xc.reshape(B, 2, HC, HEAD, H_DIM)               # b blk p h f
        xc = np.ascontiguousarray(xc.transpose(1, 4, 2, 0, 3))  # blk f p b h
        xc = xc.reshape(128, HC * 128)
        mc = M[c * S_CORE:(c + 1) * S_CORE]                  # [512, 64, 64]
        mc = mc.reshape(2, HC, H_DIM, H_DIM)                 # blk p fi fo
        mc = np.ascontiguousarray(mc.transpose(0, 2, 1, 3))  # blk fi p fo
        mc = mc.reshape(128, HC * 64)
        in_maps.append({"xin": xc, "min": mc})
    return in_maps


def _gather_output(results):
    out = np.empty((B, S, HEAD, H_DIM), dtype=np.float32)
    for c in range(N_CORES):
        yc = results[c]["yout"]                              # [128, 256*128]
        yc = yc.reshape(2, H_DIM, HC, B, HEAD)               # blk fo p b h
        yc = yc.transpose(3, 0, 2, 4, 1)                     # b blk p h fo
        out[:, c * S_CORE:(c + 1) * S_CORE] = \
            yc.reshape(B, S_CORE, HEAD, H_DIM).astype(np.float32)
    return out.reshape(B, S, D)


def kernel(x, thetas, theta_scale, r_matrix, inv_freq, pairs, **_unused):
    x = np.asarray(x, dtype=np.float32)
    M = _fold_matrices(np.asarray(thetas), np.asarray(theta_scale),
                       np.asarray(r_matrix), np.asarray(inv_freq),
                       np.asarray(pairs))
    nc = _build_nc()
    in_maps = _prep_inputs(x, M)
    res = run_bass_kernel_spmd(nc, in_maps, list(range(N_CORES)))
    return _gather_output(res.results)
